# revision 15
# baseline (speedup 1.0000x reference)
"""Fused pre-LN multi-head attention (B=4, S=2048, D=1024, H=16) on 8 trn2 cores.

Sharding: core c -> batch b = c // 2, query-half = c % 2. Each core receives
the FULL 2048-row sequence of its batch, ROTATED so that its local query half
occupies rows 0..1023 (softmax is permutation-invariant in k, so key order
does not matter). Every core redundantly computes LayerNorm + K/V projections
for the full sequence -- this removes the inter-core K/V exchange collective
entirely -- and computes Q only for its local 1024 rows. Attention (16 heads)
runs over the local queries with all 2048 keys; the output projection emits
the core's 1024 rows and the host scatters them back.

Pipeline: phase 1 streams LayerNorm + the V projection per 128-row seq tile
while building the transposed activations XNT. Phase 2 walks the 8 head
pairs; per pair it projects just that pair's K^T/Q^T slice, then runs
scores -> exp -> ctx in a kt-granular software pipeline (the ACT engine's
exp stream is the phase's critical resource; PE fills its slack with the
next pair's projections). The softmax denominator comes from a ones-column
appended to V inside the ctx matmul. LayerNorm gamma/beta and the
1/sqrt(head_dim) scale are folded into the host-pre-transposed bf16 weights.
"""

import numpy as np
import ml_dtypes

import concourse.bass as bass
import concourse.mybir as mybir
import concourse.tile as tile
from concourse import bacc
from concourse.bass_utils import run_bass_kernel_spmd

F32 = mybir.dt.float32
BF16 = mybir.dt.bfloat16

B, S, D = 4, 2048, 1024
H, HD = 16, 64
EPS = 1e-6
P = 128
NDT = D // P          # 8  d-tiles
NST = S // P          # 16 seq tiles (full sequence, every core)
QROWS = S // 2        # 1024 local query rows per core
NQT = QROWS // P      # 8
NCORES = 8
HP = H // 2           # 8 head pairs
VSTRIDE = HD + 1      # 65: per-head V columns incl. the ones column


def build_program():
    nc = bacc.Bacc("TRN2", target_bir_lowering=False)

    x_d = nc.dram_tensor("x", [S, D], F32, kind="ExternalInput")
    wqt_d = nc.dram_tensor("wqt", [D, D], BF16, kind="ExternalInput")
    wkt_d = nc.dram_tensor("wkt", [D, D], BF16, kind="ExternalInput")
    wvt_d = nc.dram_tensor("wvt", [D, D], BF16, kind="ExternalInput")
    wot_d = nc.dram_tensor("wot", [D, D], BF16, kind="ExternalInput")
    bq_d = nc.dram_tensor("bq", [NDT, P], F32, kind="ExternalInput")
    bk_d = nc.dram_tensor("bk", [NDT, P], F32, kind="ExternalInput")
    bv_d = nc.dram_tensor("bv", [1, D], F32, kind="ExternalInput")
    bo_d = nc.dram_tensor("bo", [1, D], F32, kind="ExternalInput")
    out_d = nc.dram_tensor("out", [QROWS, D], F32, kind="ExternalOutput")

    sub, mult, add = (
        mybir.AluOpType.subtract,
        mybir.AluOpType.mult,
        mybir.AluOpType.add,
    )
    AF = mybir.ActivationFunctionType

    with tile.TileContext(nc) as tc:
        with (
            tc.tile_pool(name="consts", bufs=1) as consts,
            tc.tile_pool(name="vp", bufs=1) as v_pool,
            tc.tile_pool(name="ctxt", bufs=1) as ct_pool,
            tc.tile_pool(name="xntp", bufs=1) as xnt_pool,
            tc.tile_pool(name="wk", bufs=1) as wk_pool,
            tc.tile_pool(name="wq", bufs=1) as wq_pool,
        ):
            eps_t = consts.tile([P, 1], F32)
            nc.vector.memset(eps_t, EPS)
            bq_t = consts.tile([P, NDT], F32)
            bk_t = consts.tile([P, NDT], F32)
            nc.sync.dma_start(out=bq_t, in_=bq_d.ap().rearrange("t p -> p t"))
            nc.sync.dma_start(out=bk_t, in_=bk_d.ap().rearrange("t p -> p t"))
            bvb = consts.tile([P, D], F32)
            nc.sync.dma_start(out=bvb, in_=bv_d.ap().to_broadcast([P, D]))

            # [p, seq_tile, head, 65]; v in cols 0:64, ones column at 64 so
            # the ctx matmul also produces the softmax denominator (row 64).
            V = v_pool.tile([P, NST, H * VSTRIDE], BF16)
            Vr = V.rearrange("p s (h e) -> p s h e", e=VSTRIDE)
            nc.vector.memset(Vr[:, :, :, HD : HD + 1], 1.0)

            # 4 chunk tiles (512 seq cols each) so transpose-DMA writers and
            # projection readers of different chunks never false-serialize.
            XNT = [
                xnt_pool.tile([P, NDT, S // 4], BF16, name=f"xnt{c}")
                for c in range(4)
            ]
            CT = ct_pool.tile([P, NDT, QROWS], BF16)
            WK = wk_pool.tile([P, NDT, D], BF16)
            WQ = wq_pool.tile([P, NDT, D], BF16)
            for _t in range(NDT):
                nc.scalar.dma_start(
                    out=WK[:, _t, :],
                    in_=wkt_d.ap().rearrange("(t p) j -> p t j", p=P)[:, _t, :],
                )
            for _t in range(NDT):
                nc.sync.dma_start(
                    out=WQ[:, _t, :],
                    in_=wqt_d.ap().rearrange("(t p) j -> p t j", p=P)[:, _t, :],
                )

            # ---- phase 1: LayerNorm + transpose + V projection (full seq) --
            with (
                tc.tile_pool(name="wv", bufs=1) as wv_pool,
                tc.tile_pool(name="xp", bufs=4) as x_pool,
                tc.tile_pool(name="xnp", bufs=4) as xn_pool,
                tc.tile_pool(name="statp", bufs=6) as stat_pool,
                tc.tile_pool(name="psum_v", bufs=2, space="PSUM") as psum_v,
            ):
                WV = wv_pool.tile([P, NDT, D], BF16)
                for _t in range(NDT):
                    nc.sync.dma_start(
                        out=WV[:, _t, :],
                        in_=wvt_d.ap().rearrange("(t p) j -> p t j", p=P)[:, _t, :],
                    )

                for s in range(NST):
                    xt = x_pool.tile([P, D], F32)
                    nc.gpsimd.dma_start(
                        out=xt, in_=x_d.ap()[s * P : (s + 1) * P, :]
                    )
                    st = stat_pool.tile([P, 2, 6], F32)
                    nc.vector.bn_stats(out=st[:, 0], in_=xt[:, 0:512])
                    nc.vector.bn_stats(out=st[:, 1], in_=xt[:, 512:1024])
                    mv = stat_pool.tile([P, 2], F32)
                    nc.vector.bn_aggr(out=mv, in_=st)
                    std = stat_pool.tile([P, 1], F32)
                    nc.scalar.activation(
                        out=std, in_=mv[:, 1:2], func=AF.Sqrt, bias=eps_t
                    )
                    rstd = stat_pool.tile([P, 1], F32)
                    nc.vector.reciprocal(out=rstd, in_=std)
                    xn = xn_pool.tile([P, D], BF16)
                    nc.vector.tensor_scalar(
                        out=xn,
                        in0=xt,
                        scalar1=mv[:, 0:1],
                        scalar2=rstd,
                        op0=sub,
                        op1=mult,
                    )
                    nc.scalar.dma_start(
                        out=XNT[s // 4][:, :, (s % 4) * P : (s % 4 + 1) * P],
                        in_=xn,
                        transpose=True,
                    )
                    # V projection for this seq tile
                    for df in range(2):
                        ps = psum_v.tile([P, 512], F32)
                        for k in range(NDT):
                            nc.tensor.matmul(
                                ps,
                                lhsT=XNT[s // 4][:, k, (s % 4) * P : (s % 4 + 1) * P],
                                rhs=WV[:, k, df * 512 : (df + 1) * 512],
                                start=(k == 0),
                                stop=(k == NDT - 1),
                            )
                        ps_h = ps.rearrange("p (h e) -> p h e", e=HD)
                        bv_h = bvb[:, df * 512 : (df + 1) * 512].rearrange(
                            "p (h e) -> p h e", e=HD
                        )
                        nc.vector.tensor_tensor(
                            out=Vr[:, s, df * 8 : (df + 1) * 8, 0:HD],
                            in0=ps_h,
                            in1=bv_h,
                            op=add,
                        )

            # ---- phase 2: per head pair, K/Q slice projection + attention --
            with (
                tc.tile_pool(name="wo", bufs=1) as wo_pool,
                tc.tile_pool(name="bobp", bufs=1) as bob_pool,
            ):
                WO = wo_pool.tile([P, NDT, D], BF16)
                for _t in range(NDT):
                    nc.sync.dma_start(
                        out=WO[:, _t, :],
                        in_=wot_d.ap().rearrange("(t p) j -> p t j", p=P)[:, _t, :],
                    )
                bob = bob_pool.tile([P, D], F32)
                nc.sync.dma_start(out=bob, in_=bo_d.ap().to_broadcast([P, D]))
                with (
                    tc.tile_pool(name="ktq", bufs=2) as ktq_pool,
                    tc.tile_pool(name="probs", bufs=8) as probs_pool,
                    tc.tile_pool(name="sep", bufs=2) as se_pool,
                    tc.tile_pool(name="psum_sc", bufs=2, space="PSUM") as psum_sc,
                    tc.tile_pool(name="psum_cx", bufs=1, space="PSUM") as psum_cx,
                ):
                    kt_tiles: dict[int, object] = {}
                    qt_tiles: dict[int, object] = {}

                    def emit_k_half(t, half):
                        # K^T projection for pair t's 128 head dims, one
                        # 1024-col half of the sequence.
                        if t not in kt_tiles:
                            kt_tiles[t] = ktq_pool.tile(
                                [P, S], BF16, name=f"ktt_{t}", tag="ktt"
                            )
                        ps = psum_sc.tile(
                            [P, QROWS], F32, name=f"kps_{t}_{half}", tag="sps"
                        )
                        for qf in range(2):
                            for k in range(NDT):
                                nc.tensor.matmul(
                                    ps[:, qf * 512 : (qf + 1) * 512],
                                    lhsT=WK[:, k, t * P : (t + 1) * P],
                                    rhs=XNT[half * 2 + qf][:, k, :],
                                    start=(k == 0),
                                    stop=(k == NDT - 1),
                                )
                        nc.vector.tensor_scalar(
                            out=kt_tiles[t][:, half * QROWS : (half + 1) * QROWS],
                            in0=ps,
                            scalar1=bk_t[:, t : t + 1],
                            scalar2=None,
                            op0=add,
                        )

                    def emit_q(t):
                        qt_tiles[t] = ktq_pool.tile(
                            [P, QROWS], BF16, name=f"qtt_{t}", tag="qtt"
                        )
                        ps = psum_sc.tile([P, QROWS], F32, name=f"qps_{t}", tag="sps")
                        for qf in range(2):
                            for k in range(NDT):
                                nc.tensor.matmul(
                                    ps[:, qf * 512 : (qf + 1) * 512],
                                    lhsT=WQ[:, k, t * P : (t + 1) * P],
                                    rhs=XNT[qf][:, k, :],
                                    start=(k == 0),
                                    stop=(k == NDT - 1),
                                )
                        nc.vector.tensor_scalar(
                            out=qt_tiles[t],
                            in0=ps,
                            scalar1=bq_t[:, t : t + 1],
                            scalar2=None,
                            op0=add,
                        )

                    emit_k_half(0, 0)
                    emit_k_half(0, 1)
                    emit_q(0)
                    for t in range(HP):
                        KTt = kt_tiles.pop(t)
                        QTt = qt_tiles.pop(t)

                        # scores -> exp -> ctx, pipelined at kt granularity.
                        # Four ctx accumulation chains (hi x qf) in four PSUM
                        # banks; each probs tile is consumed immediately.
                        cps_all = [
                            [
                                psum_cx.tile(
                                    [VSTRIDE, 512],
                                    F32,
                                    name=f"cps_{t}_{hi}_{qf}",
                                    tag=f"cps_{hi}_{qf}",
                                )
                                for qf in range(2)
                            ]
                            for hi in range(2)
                        ]
                        for kt in range(NST):
                            # pipeline the next pair's K/Q projection into
                            # this pair's PE slack (ACT is busy with exp)
                            if t + 1 < HP:
                                if kt == 5:
                                    emit_k_half(t + 1, 0)
                                elif kt == 9:
                                    emit_k_half(t + 1, 1)
                                elif kt == 13:
                                    emit_q(t + 1)
                            pts = []
                            for hi in range(2):
                                off = hi * HD
                                sps = psum_sc.tile(
                                    [P, QROWS],
                                    F32,
                                    name=f"sps_{t}_{kt}_{hi}",
                                    tag="sps",
                                )
                                for qf in range(2):
                                    nc.tensor.matmul(
                                        sps[:, qf * 512 : (qf + 1) * 512],
                                        lhsT=KTt[
                                            off : off + HD, kt * P : (kt + 1) * P
                                        ],
                                        rhs=QTt[
                                            off : off + HD, qf * 512 : (qf + 1) * 512
                                        ],
                                        start=True,
                                        stop=True,
                                        tile_position=(off, 0),
                                    )
                                pt = probs_pool.tile([P, QROWS], BF16)
                                nc.scalar.activation(out=pt, in_=sps, func=AF.Exp)
                                pts.append(pt)
                            for hi in range(2):
                                h = 2 * t + hi
                                for qf in range(2):
                                    nc.tensor.matmul(
                                        cps_all[hi][qf],
                                        lhsT=Vr[:, kt, h, :],
                                        rhs=pts[hi][:, qf * 512 : (qf + 1) * 512],
                                        start=(kt == 0),
                                        stop=(kt == NST - 1),
                                    )
                        for qf in range(2):
                            for hi in range(2):
                                cps = cps_all[hi][qf]
                                # rows 0..63 = unnormalized ctx, row 64 = sum
                                se = se_pool.tile([P, 512], F32, tag="se")
                                nc.vector.reciprocal(
                                    out=se[HD : HD + 1, :],
                                    in_=cps[HD : HD + 1, :],
                                )
                                # partition_broadcast reads partition 0 only:
                                # shift the denominator row up first via DMA.
                                se0 = se_pool.tile([1, 512], F32, tag="se0")
                                nc.sync.dma_start(out=se0, in_=se[HD : HD + 1, :])
                                seb = se_pool.tile([P, 512], F32, tag="seb")
                                nc.gpsimd.partition_broadcast(seb[0:HD, :], se0)
                                if hi == 0:
                                    nc.vector.tensor_tensor(
                                        out=CT[0:HD, t, qf * 512 : (qf + 1) * 512],
                                        in0=cps[0:HD, :],
                                        in1=seb[0:HD, :],
                                        op=mult,
                                    )
                                else:
                                    tmp = se_pool.tile([HD, 512], BF16, tag="ctmp")
                                    nc.vector.tensor_tensor(
                                        out=tmp,
                                        in0=cps[0:HD, :],
                                        in1=seb[0:HD, :],
                                        op=mult,
                                    )
                                    # partition shift 0..63 -> 64..127 via DMA
                                    nc.gpsimd.dma_start(
                                        out=CT[HD:P, t, qf * 512 : (qf + 1) * 512],
                                        in_=tmp,
                                    )

                # ---- output projection ------------------------------------
                with (
                    tc.tile_pool(name="osb", bufs=3) as osb_pool,
                    tc.tile_pool(name="psum_out", bufs=2, space="PSUM") as psum_out,
                ):
                    for qt in range(NQT):
                        ot = osb_pool.tile([P, D], F32)
                        for jf in range(2):
                            ps = psum_out.tile([P, 512], F32)
                            for i in range(NDT):
                                nc.tensor.matmul(
                                    ps,
                                    lhsT=CT[:, i, qt * P : (qt + 1) * P],
                                    rhs=WO[:, i, jf * 512 : (jf + 1) * 512],
                                    start=(i == 0),
                                    stop=(i == NDT - 1),
                                )
                            nc.vector.tensor_tensor(
                                out=ot[:, jf * 512 : (jf + 1) * 512],
                                in0=ps,
                                in1=bob[:, jf * 512 : (jf + 1) * 512],
                                op=add,
                            )
                        nc.scalar.dma_start(
                            out=out_d.ap()[qt * P : (qt + 1) * P, :], in_=ot
                        )

    nc.compile()
    return nc


_NC_CACHE = None


def _get_program():
    global _NC_CACHE
    if _NC_CACHE is None:
        _NC_CACHE = build_program()
    return _NC_CACHE


def _prep_host(x, ln_gamma, ln_beta, Wq, bq, Wk, bk, Wv, bv, Wo, bo):
    bf16 = ml_dtypes.bfloat16
    g = np.asarray(ln_gamma, np.float64)
    be = np.asarray(ln_beta, np.float64)
    scale = 1.0 / np.sqrt(np.float64(HD))

    def fold(W, b, s=1.0):
        W = np.asarray(W, np.float64)
        b = np.asarray(b, np.float64)
        W_eff = W * g[None, :] * s
        b_eff = (b + W @ be) * s
        wt = np.ascontiguousarray(W_eff.T).astype(bf16)
        return wt, b_eff.astype(np.float32)

    wqt, bq_e = fold(Wq, bq, scale)
    wkt, bk_e = fold(Wk, bk)
    wvt, bv_e = fold(Wv, bv)
    wot = np.ascontiguousarray(np.asarray(Wo, np.float64).T).astype(bf16)
    bo_e = np.asarray(bo, np.float32)

    shared = {
        "wqt": wqt,
        "wkt": wkt,
        "wvt": wvt,
        "wot": wot,
        "bq": bq_e.reshape(NDT, P),
        "bk": bk_e.reshape(NDT, P),
        "bv": bv_e.reshape(1, D).astype(np.float32),
        "bo": bo_e.reshape(1, D),
    }
    x = np.asarray(x, np.float32)
    in_maps = []
    for c in range(NCORES):
        b_idx, half = c // 2, c % 2
        # rotate so the local query half sits in rows 0..QROWS-1
        x_rot = np.concatenate(
            [x[b_idx, half * QROWS :], x[b_idx, : half * QROWS]], axis=0
        )
        in_maps.append({"x": np.ascontiguousarray(x_rot), **shared})
    return in_maps


def kernel(x, ln_gamma, ln_beta, Wq, bq, Wk, bk, Wv, bv, Wo, bo):
    nc = _get_program()
    in_maps = _prep_host(x, ln_gamma, ln_beta, Wq, bq, Wk, bk, Wv, bv, Wo, bo)
    res = run_bass_kernel_spmd(nc, in_maps, core_ids=list(range(NCORES)))
    out = np.empty((B, S, D), np.float32)
    for c in range(NCORES):
        b_idx, half = c // 2, c % 2
        out[b_idx, half * QROWS : (half + 1) * QROWS] = res.results[c]["out"]
    return out


if __name__ == "__main__":
    build_program()
    print("program built OK")


# revision 21
# speedup vs baseline: 1.1018x; 1.1018x over previous
"""Fused pre-LN multi-head attention (B=4, S=2048, D=1024, H=16) on 8 trn2 cores.

Sharding: core c -> batch b = c // 2, query-half = c % 2. Each core receives
the FULL 2048-row sequence of its batch, ROTATED so that its local query half
occupies rows 0..1023 (softmax is permutation-invariant in k, so key order
does not matter). Every core redundantly computes LayerNorm + K/V projections
for the full sequence -- this removes the inter-core K/V exchange collective
entirely -- and computes Q only for its local 1024 rows. Attention (16 heads)
runs over the local queries with all 2048 keys; the output projection emits
the core's 1024 rows and the host scatters them back.

LayerNorm gamma/beta and the 1/sqrt(head_dim) scale are folded into the
(host-pre-transposed, bf16) projection weights. Softmax skips max-subtraction
(scores are O(1) by construction); the denominator comes from a ones-column
appended to V inside the probs @ V matmul.
"""

import numpy as np
import ml_dtypes

import concourse.bass as bass
import concourse.mybir as mybir
import concourse.tile as tile
from concourse import bacc
from concourse.bass_utils import run_bass_kernel_spmd

F32 = mybir.dt.float32
BF16 = mybir.dt.bfloat16

B, S, D = 4, 2048, 1024
H, HD = 16, 64
EPS = 1e-6
P = 128
NDT = D // P          # 8  d-tiles
NST = S // P          # 16 seq tiles (full sequence, every core)
QROWS = S // 2        # 1024 local query rows per core
NQT = QROWS // P      # 8
NCH = 4               # 512-row chunks for the projection pipeline
TPC = NST // NCH      # 4 seq tiles per chunk
NCORES = 8
HP = H // 2           # 8 head pairs
VSTRIDE = HD + 1      # 65: per-head V columns incl. the ones column


def build_program():
    nc = bacc.Bacc("TRN2", target_bir_lowering=False)

    x_d = nc.dram_tensor("x", [S, D], F32, kind="ExternalInput")
    wqt_d = nc.dram_tensor("wqt", [D, D], BF16, kind="ExternalInput")
    wkt_d = nc.dram_tensor("wkt", [D, D], BF16, kind="ExternalInput")
    wvt_d = nc.dram_tensor("wvt", [D, D], BF16, kind="ExternalInput")
    wot_d = nc.dram_tensor("wot", [D, D], BF16, kind="ExternalInput")
    bq_d = nc.dram_tensor("bq", [NDT, P], F32, kind="ExternalInput")
    bk_d = nc.dram_tensor("bk", [NDT, P], F32, kind="ExternalInput")
    bv_d = nc.dram_tensor("bv", [1, D], F32, kind="ExternalInput")
    bo_d = nc.dram_tensor("bo", [1, D], F32, kind="ExternalInput")
    out_d = nc.dram_tensor("out", [QROWS, D], F32, kind="ExternalOutput")

    sub, mult, add = (
        mybir.AluOpType.subtract,
        mybir.AluOpType.mult,
        mybir.AluOpType.add,
    )
    AF = mybir.ActivationFunctionType

    with tile.TileContext(nc) as tc:
        with (
            tc.tile_pool(name="consts", bufs=1) as consts,
            tc.tile_pool(name="qt", bufs=1) as qt_pool,
            tc.tile_pool(name="kt", bufs=1) as kt_pool,
            tc.tile_pool(name="vp", bufs=1) as v_pool,
            tc.tile_pool(name="ctxt", bufs=1) as ct_pool,
        ):
            eps_t = consts.tile([P, 1], F32)
            nc.vector.memset(eps_t, EPS)
            bq_t = consts.tile([P, NDT], F32)
            bk_t = consts.tile([P, NDT], F32)
            nc.sync.dma_start(out=bq_t, in_=bq_d.ap().rearrange("t p -> p t"))
            nc.sync.dma_start(out=bk_t, in_=bk_d.ap().rearrange("t p -> p t"))
            bvb = consts.tile([P, D], F32)
            nc.sync.dma_start(out=bvb, in_=bv_d.ap().to_broadcast([P, D]))

            # [p, seq_tile, head, 65]; v in cols 0:64, ones column at 64 so
            # the ctx matmul also produces the softmax denominator (row 64).
            V = v_pool.tile([P, NST, H * VSTRIDE], BF16)
            Vr = V.rearrange("p s (h e) -> p s h e", e=VSTRIDE)
            nc.vector.memset(Vr[:, :, :, HD : HD + 1], 1.0)

            QT = qt_pool.tile([P, NDT, QROWS], BF16)
            KT = kt_pool.tile([P, NDT, S], BF16)
            CT = ct_pool.tile([P, NDT, QROWS], BF16)

            # ---- LayerNorm + transpose + V/K/Q projections (full seq) ----
            with (
                tc.tile_pool(name="wq", bufs=1) as wq_pool,
                tc.tile_pool(name="wk", bufs=1) as wk_pool,
                tc.tile_pool(name="wv", bufs=1) as wv_pool,
                tc.tile_pool(name="xp", bufs=6) as x_pool,
                tc.tile_pool(name="xnp", bufs=6) as xn_pool,
                tc.tile_pool(name="xntp", bufs=2) as xnt_pool,
                tc.tile_pool(name="statp", bufs=20) as stat_pool,
                tc.tile_pool(name="psum_proj", bufs=2, space="PSUM") as psum_proj,
            ):
                WQ = wq_pool.tile([P, NDT, D], BF16)
                WK = wk_pool.tile([P, NDT, D], BF16)
                WV = wv_pool.tile([P, NDT, D], BF16)
                for _t in range(NDT):
                    nc.sync.dma_start(
                        out=WV[:, _t, :],
                        in_=wvt_d.ap().rearrange("(t p) j -> p t j", p=P)[:, _t, :],
                    )
                for _t in range(NDT):
                    nc.sync.dma_start(
                        out=WK[:, _t, :],
                        in_=wkt_d.ap().rearrange("(t p) j -> p t j", p=P)[:, _t, :],
                    )
                for _t in range(NDT):
                    nc.sync.dma_start(
                        out=WQ[:, _t, :],
                        in_=wqt_d.ap().rearrange("(t p) j -> p t j", p=P)[:, _t, :],
                    )

                for ch in range(NCH):
                    XC = xnt_pool.tile([P, NDT, TPC * P], BF16)
                    for j in range(TPC):
                        s = ch * TPC + j
                        xt = x_pool.tile([P, D], F32)
                        nc.gpsimd.dma_start(
                            out=xt, in_=x_d.ap()[s * P : (s + 1) * P, :]
                        )
                        st = stat_pool.tile([P, 2, 6], F32)
                        nc.vector.bn_stats(out=st[:, 0], in_=xt[:, 0:512])
                        nc.vector.bn_stats(out=st[:, 1], in_=xt[:, 512:1024])
                        mv = stat_pool.tile([P, 2], F32)
                        nc.vector.bn_aggr(out=mv, in_=st)
                        std = stat_pool.tile([P, 1], F32)
                        nc.scalar.activation(
                            out=std, in_=mv[:, 1:2], func=AF.Sqrt, bias=eps_t
                        )
                        rstd = stat_pool.tile([P, 1], F32)
                        nc.vector.reciprocal(out=rstd, in_=std)
                        xn = xn_pool.tile([P, D], BF16)
                        nc.vector.tensor_scalar(
                            out=xn,
                            in0=xt,
                            scalar1=mv[:, 0:1],
                            scalar2=rstd,
                            op0=sub,
                            op1=mult,
                        )
                        nc.scalar.dma_start(
                            out=XC[:, :, j * P : (j + 1) * P],
                            in_=xn,
                            transpose=True,
                        )
                        # V projection for this seq tile
                        for df in range(2):
                            ps = psum_proj.tile([P, 512], F32)
                            for k in range(NDT):
                                nc.tensor.matmul(
                                    ps,
                                    lhsT=XC[:, k, j * P : (j + 1) * P],
                                    rhs=WV[:, k, df * 512 : (df + 1) * 512],
                                    start=(k == 0),
                                    stop=(k == NDT - 1),
                                )
                            ps_h = ps.rearrange("p (h e) -> p h e", e=HD)
                            bv_h = bvb[:, df * 512 : (df + 1) * 512].rearrange(
                                "p (h e) -> p h e", e=HD
                            )
                            nc.vector.tensor_tensor(
                                out=Vr[:, s, df * 8 : (df + 1) * 8, 0:HD],
                                in0=ps_h,
                                in1=bv_h,
                                op=add,
                            )
                    # K^T for this 512-column chunk
                    for i in range(NDT):
                        ps = psum_proj.tile([P, 512], F32)
                        for k in range(NDT):
                            nc.tensor.matmul(
                                ps,
                                lhsT=WK[:, k, i * P : (i + 1) * P],
                                rhs=XC[:, k, :],
                                start=(k == 0),
                                stop=(k == NDT - 1),
                            )
                        nc.vector.tensor_scalar(
                            out=KT[:, i, ch * 512 : (ch + 1) * 512],
                            in0=ps,
                            scalar1=bk_t[:, i : i + 1],
                            scalar2=None,
                            op0=add,
                        )
                    # Q^T: local query rows are seq tiles 0..7 (chunks 0, 1)
                    if ch < 2:
                        for i in range(NDT):
                            ps = psum_proj.tile([P, 512], F32)
                            for k in range(NDT):
                                nc.tensor.matmul(
                                    ps,
                                    lhsT=WQ[:, k, i * P : (i + 1) * P],
                                    rhs=XC[:, k, :],
                                    start=(k == 0),
                                    stop=(k == NDT - 1),
                                )
                            nc.vector.tensor_scalar(
                                out=QT[:, i, ch * 512 : (ch + 1) * 512],
                                in0=ps,
                                scalar1=bq_t[:, i : i + 1],
                                scalar2=None,
                                op0=add,
                            )

            # ---- attention (WO prefetched so out-proj starts instantly) --
            with (
                tc.tile_pool(name="wo", bufs=1) as wo_pool,
                tc.tile_pool(name="bobp", bufs=1) as bob_pool,
            ):
                WO = wo_pool.tile([P, NDT, D], BF16)
                for _t in range(NDT):
                    nc.sync.dma_start(
                        out=WO[:, _t, :],
                        in_=wot_d.ap().rearrange("(t p) j -> p t j", p=P)[:, _t, :],
                    )
                bob = bob_pool.tile([P, D], F32)
                nc.sync.dma_start(out=bob, in_=bo_d.ap().to_broadcast([P, D]))
                with (
                    tc.tile_pool(name="probs", bufs=34) as probs_pool,
                    tc.tile_pool(name="sep", bufs=2) as se_pool,
                    tc.tile_pool(name="psum_sc", bufs=2, space="PSUM") as psum_sc,
                    tc.tile_pool(name="psum_cx", bufs=1, space="PSUM") as psum_cx,
                ):
                    for t in range(HP):
                        probs = [[None] * NST for _ in range(2)]
                        for kt in range(NST):
                            for hi in range(2):
                                off = hi * HD
                                sps = psum_sc.tile([P, QROWS], F32)
                                for qf in range(QROWS // 512):
                                    nc.tensor.matmul(
                                        sps[:, qf * 512 : (qf + 1) * 512],
                                        lhsT=KT[
                                            off : off + HD, t, kt * P : (kt + 1) * P
                                        ],
                                        rhs=QT[
                                            off : off + HD, t, qf * 512 : (qf + 1) * 512
                                        ],
                                        start=True,
                                        stop=True,
                                        tile_position=(off, 0),
                                    )
                                pt = probs_pool.tile([P, QROWS], BF16)
                                nc.scalar.activation(out=pt, in_=sps, func=AF.Exp)
                                probs[hi][kt] = pt
                        # Four accumulation chains (hi x qf) interleaved at kt
                        # granularity across four PSUM banks, so each probs
                        # tile is released right after its kt pass and the
                        # next pair's scores can proceed.
                        cps_all = [
                            [
                                psum_cx.tile(
                                    [VSTRIDE, 512],
                                    F32,
                                    name=f"cps_{t}_{hi}_{qf}",
                                    tag=f"cps_{hi}_{qf}",
                                )
                                for qf in range(2)
                            ]
                            for hi in range(2)
                        ]
                        for kt in range(NST):
                            for hi in range(2):
                                h = 2 * t + hi
                                for qf in range(QROWS // 512):
                                    nc.tensor.matmul(
                                        cps_all[hi][qf],
                                        lhsT=Vr[:, kt, h, :],
                                        rhs=probs[hi][kt][:, qf * 512 : (qf + 1) * 512],
                                        start=(kt == 0),
                                        stop=(kt == NST - 1),
                                    )
                        for qf in range(QROWS // 512):
                            for hi in range(2):
                                cps = cps_all[hi][qf]
                                # rows 0..63 = unnormalized ctx, row 64 = sum(exp)
                                se = se_pool.tile([P, 512], F32, tag="se")
                                nc.vector.reciprocal(
                                    out=se[HD : HD + 1, :],
                                    in_=cps[HD : HD + 1, :],
                                )
                                # HW partition_broadcast only reads partition 0:
                                # shift the denominator row down first via DMA.
                                se0 = se_pool.tile([1, 512], F32, tag="se0")
                                nc.sync.dma_start(out=se0, in_=se[HD : HD + 1, :])
                                seb = se_pool.tile([P, 512], F32, tag="seb")
                                nc.gpsimd.partition_broadcast(seb[0:HD, :], se0)
                                if hi == 0:
                                    nc.vector.tensor_tensor(
                                        out=CT[0:HD, t, qf * 512 : (qf + 1) * 512],
                                        in0=cps[0:HD, :],
                                        in1=seb[0:HD, :],
                                        op=mult,
                                    )
                                else:
                                    tmp = se_pool.tile([HD, 512], BF16, tag="ctmp")
                                    nc.vector.tensor_tensor(
                                        out=tmp,
                                        in0=cps[0:HD, :],
                                        in1=seb[0:HD, :],
                                        op=mult,
                                    )
                                    # partition shift 0..63 -> 64..127 via DMA
                                    nc.gpsimd.dma_start(
                                        out=CT[HD:P, t, qf * 512 : (qf + 1) * 512],
                                        in_=tmp,
                                    )

                # ---- output projection ------------------------------------
                with (
                    tc.tile_pool(name="osb", bufs=3) as osb_pool,
                    tc.tile_pool(name="psum_out", bufs=2, space="PSUM") as psum_out,
                ):
                    for qt in range(NQT):
                        ot = osb_pool.tile([P, D], F32)
                        for jf in range(2):
                            ps = psum_out.tile([P, 512], F32)
                            for i in range(NDT):
                                nc.tensor.matmul(
                                    ps,
                                    lhsT=CT[:, i, qt * P : (qt + 1) * P],
                                    rhs=WO[:, i, jf * 512 : (jf + 1) * 512],
                                    start=(i == 0),
                                    stop=(i == NDT - 1),
                                )
                            nc.vector.tensor_tensor(
                                out=ot[:, jf * 512 : (jf + 1) * 512],
                                in0=ps,
                                in1=bob[:, jf * 512 : (jf + 1) * 512],
                                op=add,
                            )
                        nc.scalar.dma_start(
                            out=out_d.ap()[qt * P : (qt + 1) * P, :], in_=ot
                        )

    nc.compile()
    return nc


_NC_CACHE = None


def _get_program():
    global _NC_CACHE
    if _NC_CACHE is None:
        _NC_CACHE = build_program()
    return _NC_CACHE


def _prep_host(x, ln_gamma, ln_beta, Wq, bq, Wk, bk, Wv, bv, Wo, bo):
    bf16 = ml_dtypes.bfloat16
    g = np.asarray(ln_gamma, np.float64)
    be = np.asarray(ln_beta, np.float64)
    scale = 1.0 / np.sqrt(np.float64(HD))

    def fold(W, b, s=1.0):
        W = np.asarray(W, np.float64)
        b = np.asarray(b, np.float64)
        W_eff = W * g[None, :] * s
        b_eff = (b + W @ be) * s
        wt = np.ascontiguousarray(W_eff.T).astype(bf16)
        return wt, b_eff.astype(np.float32)

    wqt, bq_e = fold(Wq, bq, scale)
    wkt, bk_e = fold(Wk, bk)
    wvt, bv_e = fold(Wv, bv)
    wot = np.ascontiguousarray(np.asarray(Wo, np.float64).T).astype(bf16)
    bo_e = np.asarray(bo, np.float32)

    shared = {
        "wqt": wqt,
        "wkt": wkt,
        "wvt": wvt,
        "wot": wot,
        "bq": bq_e.reshape(NDT, P),
        "bk": bk_e.reshape(NDT, P),
        "bv": bv_e.reshape(1, D).astype(np.float32),
        "bo": bo_e.reshape(1, D),
    }
    x = np.asarray(x, np.float32)
    in_maps = []
    for c in range(NCORES):
        b_idx, half = c // 2, c % 2
        # rotate so the local query half sits in rows 0..QROWS-1
        x_rot = np.concatenate(
            [x[b_idx, half * QROWS :], x[b_idx, : half * QROWS]], axis=0
        )
        in_maps.append({"x": np.ascontiguousarray(x_rot), **shared})
    return in_maps


def kernel(x, ln_gamma, ln_beta, Wq, bq, Wk, bk, Wv, bv, Wo, bo):
    nc = _get_program()
    in_maps = _prep_host(x, ln_gamma, ln_beta, Wq, bq, Wk, bk, Wv, bv, Wo, bo)
    res = run_bass_kernel_spmd(nc, in_maps, core_ids=list(range(NCORES)))
    out = np.empty((B, S, D), np.float32)
    for c in range(NCORES):
        b_idx, half = c // 2, c % 2
        out[b_idx, half * QROWS : (half + 1) * QROWS] = res.results[c]["out"]
    return out


if __name__ == "__main__":
    build_program()
    print("program built OK")
